# revision 11
# baseline (speedup 1.0000x reference)
"""LogSigRNN generator kernel for Trainium2 (8 NeuronCores, data-parallel).

Self-contained: hardcodes problem shapes (B=4096, L=1000, d=3, hidden=256,
n_lags=64) and the sharding (batch / 8 cores). Computes the full forward:
Brownian path + depth-2 log-signature features + latched RNN, all on device.
"""
import numpy as np
from contextlib import ExitStack

import concourse.bass as bass
import concourse.tile as tile
from concourse import bacc, mybir
from concourse.bass_utils import run_bass_kernel_spmd
from concourse.masks import make_identity

F32 = mybir.dt.float32
F32R = mybir.dt.float32r
AF = mybir.ActivationFunctionType
ALU = mybir.AluOpType

LEN_NOISE = 1000
LEN_INTERVAL_U = 50
D = 3
HID = 256
LS = 6            # 3 + 3 logsig channels
ODIM = 3
NCORES = 8
PAIRS = [(0, 1), (0, 2), (1, 2)]


def _schedule(n_lags):
    """Interval indices + latch schedule (mirrors the model definition)."""
    tb = np.linspace(0.0, 1.0, LEN_NOISE)
    tu = tb[::LEN_INTERVAL_U]
    tt = np.linspace(0.0, 1.0, n_lags)
    ind_low, ind_max, u_list = [], [], []
    last_u = -1.0
    for t in tt[1:]:
        u = tu[tu < t].max()
        lo = int(np.nonzero(tb <= u)[0].max())
        if u != last_u:
            u_list.append(u)
            last_u = u
        hi = int(np.nonzero(tb <= t)[0].max())
        ind_low.append(lo)
        ind_max.append(hi)
    u_list.append(tt[-1])
    latch = np.zeros(n_lags, dtype=bool)
    q = list(u_list)
    for i, t in enumerate(tt):
        if q and t >= q[0]:
            q.pop(0)
            latch[i] = True
    return np.asarray(ind_low), np.asarray(ind_max), latch


def _constants(n_lags):
    """Host-side constant matrices fed to the device."""
    K = n_lags - 1
    ind_low, ind_max, latch = _schedule(n_lags)
    NZ = LEN_NOISE - 1                      # 999 usable z rows (1..999)
    NT = (NZ + 127) // 128                  # 8 L-tiles
    cnt = [min(128, NZ - 128 * j) for j in range(NT)]

    TS = np.zeros((128, 128), np.float32)   # strict lower triangle
    for k in range(128):
        TS[k, k + 1:] = 1.0
    ONES = np.ones((128, 128), np.float32)

    PSEL = np.zeros((128, NT * K), np.float32)
    SSEL = np.zeros((128, NT * K), np.float32)
    for j in range(NT):
        for p in range(cnt[j]):
            zrow = 1 + 128 * j + p          # z index for zs row p of tile j
            l = 128 * j + p                 # anti/bp index
            for k in range(K):
                if zrow <= ind_max[k]:
                    PSEL[p, K * j + k] = 1.0
                if ind_low[k] <= l < ind_max[k]:
                    SSEL[p, K * j + k] = 1.0

    return dict(TS=TS, ONES=ONES, PSEL=PSEL, SSEL=SSEL,
                latch=latch, cnt=cnt, NT=NT, K=K)


_CACHE = {}


def _build(NB, n_lags, alpha1, alpha2):
    """Build + finalize the bass module for one core's shard of size NB."""
    C = _constants(n_lags)
    NT, K, cnt = C["NT"], C["K"], C["cnt"]
    sqrt_dt = float(np.sqrt(1.0 / (LEN_NOISE - 1)))

    nc = bacc.Bacc("TRN2", target_bir_lowering=False, debug=False)

    d_z = nc.dram_tensor("z", [NB, LEN_NOISE * D], F32, kind="ExternalInput")
    d_W1 = nc.dram_tensor("W1", [HID + LS, HID], F32R, kind="ExternalInput")
    d_W2 = nc.dram_tensor("W2", [HID, HID], F32R, kind="ExternalInput")
    d_W3 = nc.dram_tensor("W3", [HID, HID], F32R, kind="ExternalInput")
    d_Wo = nc.dram_tensor("Wo", [HID, ODIM], F32R, kind="ExternalInput")
    d_b1 = nc.dram_tensor("b1", [HID, 1], F32, kind="ExternalInput")
    d_b2 = nc.dram_tensor("b2", [HID, 1], F32, kind="ExternalInput")
    d_b3 = nc.dram_tensor("b3", [HID, 1], F32, kind="ExternalInput")
    d_TS = nc.dram_tensor("TS", [128, 128], F32R, kind="ExternalInput")
    d_ONES = nc.dram_tensor("ONES", [128, 128], F32R, kind="ExternalInput")
    d_PSEL = nc.dram_tensor("PSEL", [128, NT * K], F32R, kind="ExternalInput")
    d_SSEL = nc.dram_tensor("SSEL", [128, NT * K], F32R, kind="ExternalInput")
    d_out = nc.dram_tensor("out", [NB, n_lags * ODIM], F32, kind="ExternalOutput")
    d_sig = nc.dram_tensor("sigsc", [LS, n_lags * NB], F32R)  # internal scratch

    NBT = NB // 128  # batch tiles
    NO2 = (n_lags + 1) // 2

    with tile.TileContext(nc) as tc, ExitStack() as ctx:
        cons = ctx.enter_context(tc.tile_pool(name="cons", bufs=1))
        outp = ctx.enter_context(tc.tile_pool(name="outp", bufs=1))

        # ---- constants / weights to SBUF ----
        sTS = cons.tile([128, 128], F32R); nc.sync.dma_start(sTS[:], d_TS[:])
        sONES = cons.tile([128, 128], F32R); nc.sync.dma_start(sONES[:], d_ONES[:])
        sPSEL = cons.tile([128, NT * K], F32R); nc.sync.dma_start(sPSEL[:], d_PSEL[:])
        sSSEL = cons.tile([128, NT * K], F32R); nc.sync.dma_start(sSSEL[:], d_SSEL[:])
        ident = cons.tile([128, 128], F32)
        make_identity(nc, ident[:])

        W1a = cons.tile([128, HID], F32R); nc.sync.dma_start(W1a[:], d_W1[0:128, :])
        W1b = cons.tile([128, HID], F32R); nc.sync.dma_start(W1b[:], d_W1[128:256, :])
        W1s = cons.tile([LS, HID], F32R); nc.sync.dma_start(W1s[:], d_W1[256:262, :])
        W2a = cons.tile([128, HID], F32R); nc.sync.dma_start(W2a[:], d_W2[0:128, :])
        W2b = cons.tile([128, HID], F32R); nc.sync.dma_start(W2b[:], d_W2[128:256, :])
        W3a = cons.tile([128, HID], F32R); nc.sync.dma_start(W3a[:], d_W3[0:128, :])
        W3b = cons.tile([128, HID], F32R); nc.sync.dma_start(W3b[:], d_W3[128:256, :])
        Woa = cons.tile([128, ODIM], F32R); nc.sync.dma_start(Woa[:], d_Wo[0:128, :])
        Wob = cons.tile([128, ODIM], F32R); nc.sync.dma_start(Wob[:], d_Wo[128:256, :])
        b1a = cons.tile([128, 1], F32); nc.sync.dma_start(b1a[:], d_b1[0:128, :])
        b1b = cons.tile([128, 1], F32); nc.sync.dma_start(b1b[:], d_b1[128:256, :])
        b2a = cons.tile([128, 1], F32); nc.sync.dma_start(b2a[:], d_b2[0:128, :])
        b2b = cons.tile([128, 1], F32); nc.sync.dma_start(b2b[:], d_b2[128:256, :])
        b3a = cons.tile([128, 1], F32); nc.sync.dma_start(b3a[:], d_b3[0:128, :])
        b3b = cons.tile([128, 1], F32); nc.sync.dma_start(b3b[:], d_b3[128:256, :])

        out_all = [outp.tile([ODIM * NO2, NB], F32, tag=f"oall{h}", name=f"oall{h}")
                   for h in range(2)]

        eng_alt = [nc.scalar, nc.vector]

        def copy_out(eng, dst, src):
            if eng is nc.scalar:
                eng.activation(dst, src, AF.Copy)
            else:
                eng.tensor_copy(dst, src)

        # =================== preprocessing ===================
        with tc.tile_pool(name="zs", bufs=1) as zsp, \
             tc.tile_pool(name="bp", bufs=1) as bpp, \
             tc.tile_pool(name="segsb", bufs=1) as segsb_p, \
             tc.tile_pool(name="znat", bufs=2) as znat_p, \
             tc.tile_pool(name="prod", bufs=6) as prod_p, \
             tc.tile_pool(name="tp_ps", bufs=4, space="PSUM") as tp_ps, \
             tc.tile_pool(name="bp_ps", bufs=2, space="PSUM") as bp_ps, \
             tc.tile_pool(name="seg_ps", bufs=2, space="PSUM") as seg_ps:

            zs = [[zsp.tile([128, NB], F32R, tag=f"zs{c}_{j}", name=f"zs{c}_{j}")
                   for j in range(NT)]
                  for c in range(D)]
            bp = [[bpp.tile([128, NB], F32R, tag=f"bp{c}_{j}", name=f"bp{c}_{j}")
                   for j in range(NT)]
                  for c in range(D)]


            # ---- load z natural + transpose to [L, batch] per channel ----
            for bt in range(NBT):
                znat = znat_p.tile([128, LEN_NOISE * D], F32)
                nc.sync.dma_start(znat[:], d_z[128 * bt:128 * (bt + 1), :])
                for c in range(D):
                    for j in range(NT):
                        ps = tp_ps.tile([128, 128], F32, tag="tp")
                        off = D * (1 + 128 * j) + c
                        nc.tensor.transpose(
                            ps[0:cnt[j], :],
                            znat[:, off:off + D * (cnt[j] - 1) + 1:D],
                            ident[:])
                        copy_out(eng_alt[(c + j) % 2],
                                 zs[c][j][0:cnt[j], 128 * bt:128 * (bt + 1)],
                                 ps[0:cnt[j], :])

            # ---- bp[j] = colsum(zs[<j]) + strict_cumsum(zs[j]) ----
            for c in range(D):
                for j in range(NT):
                    ps = bp_ps.tile([128, NB], F32, tag="bpps")
                    for jp in range(j):
                        nc.tensor.matmul(ps[:], sONES[0:cnt[jp], :],
                                         zs[c][jp][0:cnt[jp], :],
                                         start=(jp == 0), stop=False)
                    nc.tensor.matmul(ps[:], sTS[0:cnt[j], :], zs[c][j][0:cnt[j], :],
                                     start=(j == 0), stop=True)
                    copy_out(eng_alt[j % 2], bp[c][j][:], ps[:])

            # ---- level-1 + level-2 segment sums -> sig scratch in DRAM ----
            zf32 = cons.tile([128, NB], F32)
            nc.vector.memset(zf32[:], 0.0)
            zero6 = cons.tile([LS, NB], F32R)
            nc.scalar.activation(zero6[:], zf32[0:LS, :], AF.Copy)
            nc.sync.dma_start(d_sig[:, 0:NB], zero6[:])

            for c in range(D):
                ps = seg_ps.tile([K, NB], F32, tag="seg")
                for j in range(NT):
                    nc.tensor.matmul(ps[:], sPSEL[0:cnt[j], K * j:K * (j + 1)],
                                     zs[c][j][0:cnt[j], :],
                                     start=(j == 0), stop=(j == NT - 1))
                sb = segsb_p.tile([K, NB], F32R, tag=f"seg{c}")
                nc.scalar.activation(sb[:], ps[:], AF.Copy, scale=sqrt_dt)
                nc.sync.dma_start(
                    d_sig[c:c + 1, NB:].rearrange("s (k n) -> (s k) n", k=K), sb[:])

            for ip, (a, b) in enumerate(PAIRS):
                ps = seg_ps.tile([K, NB], F32, tag="seg")
                for j in range(NT):
                    cj = cnt[j]
                    q1 = prod_p.tile([128, NB], F32R, tag="q")
                    nc.vector.tensor_mul(q1[0:cj, :], bp[a][j][0:cj, :],
                                         zs[b][j][0:cj, :])
                    q2 = prod_p.tile([128, NB], F32R, tag="q")
                    nc.vector.tensor_mul(q2[0:cj, :], bp[b][j][0:cj, :],
                                         zs[a][j][0:cj, :])
                    anti = prod_p.tile([128, NB], F32R, tag="anti")
                    nc.vector.tensor_sub(anti[0:cj, :], q1[0:cj, :], q2[0:cj, :])
                    nc.tensor.matmul(ps[:], sSSEL[0:cnt[j], K * j:K * (j + 1)],
                                     anti[0:cnt[j], :],
                                     start=(j == 0), stop=(j == NT - 1))
                sb = segsb_p.tile([K, NB], F32R, tag=f"seg{3 + ip}")
                nc.scalar.activation(sb[:], ps[:], AF.Copy,
                                     scale=0.5 * sqrt_dt * sqrt_dt)
                nc.sync.dma_start(
                    d_sig[3 + ip:4 + ip, NB:].rearrange("s (k n) -> (s k) n", k=K),
                    sb[:])

        # =================== RNN ===================
        with tc.tile_pool(name="hst", bufs=1) as hst_p, \
             tc.tile_pool(name="sigt", bufs=6) as sigt_p, \
             tc.tile_pool(name="u", bufs=4) as u_p, \
             tc.tile_pool(name="h3", bufs=6) as h3_p, \
             tc.tile_pool(name="dvet", bufs=4) as dvet_p, \
             tc.tile_pool(name="osb", bufs=4) as osb_p, \
             tc.tile_pool(name="p1", bufs=3, space="PSUM") as p1_p, \
             tc.tile_pool(name="p2", bufs=2, space="PSUM") as p2_p, \
             tc.tile_pool(name="p3", bufs=2, space="PSUM") as p3_p, \
             tc.tile_pool(name="po", bufs=1, space="PSUM") as po_p:

            zf32b = hst_p.tile([128, NB], F32, tag="zf32b")
            nc.vector.memset(zf32b[:], 0.0)
            h0a = hst_p.tile([128, NB], F32R, tag="h0a")
            h0b = hst_p.tile([128, NB], F32R, tag="h0b")
            nc.scalar.activation(h0a[:], zf32b[:], AF.Copy)
            nc.scalar.activation(h0b[:], zf32b[:], AF.Copy)
            state = (h0a, h0b)
            latch = C["latch"]

            def prelu_act(dst, psrc, bias_t, alpha):
                nc.scalar.activation(dst, psrc, AF.Prelu, bias=bias_t, alpha=alpha)

            def prelu_dve(dst, psrc, bias_t, alpha):
                t1 = dvet_p.tile([128, NB], F32, tag="t1")
                nc.vector.tensor_scalar_add(t1[:], psrc, bias_t)
                nc.vector.scalar_tensor_tensor(dst, t1[:], alpha, t1[:],
                                               op0=ALU.mult, op1=ALU.max)

            for t in range(n_lags):
                ha, hb = state
                sigt = sigt_p.tile([LS, NB], F32R, tag="sigt")
                nc.sync.dma_start(sigt[:], d_sig[:, t * NB:(t + 1) * NB])

                u1 = []
                for i in range(2):
                    ps = p1_p.tile([128, NB], F32, tag="p1")
                    cs = slice(128 * i, 128 * (i + 1))
                    nc.tensor.matmul(ps[:], W1a[:, cs], ha[:], start=True, stop=False)
                    nc.tensor.matmul(ps[:], W1b[:, cs], hb[:], start=False, stop=False)
                    nc.tensor.matmul(ps[:], W1s[:, cs], sigt[:], start=False, stop=True)
                    u = u_p.tile([128, NB], F32R, tag="u1")
                    if i == 0:
                        prelu_act(u[:], ps[:], b1a[:, 0:1], alpha1)
                    else:
                        prelu_dve(u[:], ps[:], b1b[:, 0:1], alpha1)
                    u1.append(u)

                u2 = []
                for i in range(2):
                    ps = p2_p.tile([128, NB], F32, tag="p2")
                    cs = slice(128 * i, 128 * (i + 1))
                    nc.tensor.matmul(ps[:], W2a[:, cs], u1[0][:], start=True, stop=False)
                    nc.tensor.matmul(ps[:], W2b[:, cs], u1[1][:], start=False, stop=True)
                    u = u_p.tile([128, NB], F32R, tag="u2")
                    if i == 0:
                        prelu_act(u[:], ps[:], b2a[:, 0:1], alpha2)
                    else:
                        prelu_dve(u[:], ps[:], b2b[:, 0:1], alpha2)
                    u2.append(u)

                h3 = []
                for i in range(2):
                    ps = p3_p.tile([128, NB], F32, tag="p3")
                    cs = slice(128 * i, 128 * (i + 1))
                    nc.tensor.matmul(ps[:], W3a[:, cs], u2[0][:], start=True, stop=False)
                    nc.tensor.matmul(ps[:], W3b[:, cs], u2[1][:], start=False, stop=True)
                    h = h3_p.tile([128, NB], F32R, tag="h3")
                    nc.scalar.activation(h[:], ps[:], AF.Tanh,
                                         bias=(b3a if i == 0 else b3b)[:, 0:1])
                    h3.append(h)

                ps = po_p.tile([ODIM, NB], F32, tag="po")
                nc.tensor.matmul(ps[:], Woa[:], h3[0][:], start=True, stop=False)
                nc.tensor.matmul(ps[:], Wob[:], h3[1][:], start=False, stop=True)
                osb = osb_p.tile([ODIM, NB], F32, tag="osb")
                nc.scalar.activation(osb[:], ps[:], AF.Copy)
                half, tm = divmod(t, NO2)
                nc.sync.dma_start(out_all[half][ODIM * tm:ODIM * (tm + 1), :],
                                  osb[:])

                if latch[t]:
                    state = (h3[0], h3[1])

        # ---- transpose outputs to batch-major and store ----
        with tc.tile_pool(name="ot_ps", bufs=2, space="PSUM") as ot_ps, \
             tc.tile_pool(name="otp", bufs=2) as ot_p:
            nrow = ODIM * NO2
            for bt in range(NBT):
                oT = ot_p.tile([128, ODIM * n_lags], F32, tag="oT")
                for half in range(2):
                    ps = ot_ps.tile([128, nrow], F32, tag="ot")
                    nc.tensor.transpose(ps[:, 0:nrow],
                                        out_all[half][0:nrow,
                                                      128 * bt:128 * (bt + 1)],
                                        ident[0:nrow, 0:nrow])
                    nc.scalar.activation(
                        oT[:, half * nrow:(half + 1) * nrow], ps[:, 0:nrow], AF.Copy)
                nc.sync.dma_start(d_out[128 * bt:128 * (bt + 1), :], oT[:])

    nc.finalize()
    return nc, C


def _get(NB, n_lags, alpha1, alpha2):
    key = (NB, n_lags, alpha1, alpha2)
    if key not in _CACHE:
        _CACHE[key] = _build(NB, n_lags, alpha1, alpha2)
    return _CACHE[key]


def run(trace=False, **inputs):
    z = np.ascontiguousarray(np.asarray(inputs["z"], dtype=np.float32))
    B, L, d = z.shape
    n_lags = int(np.asarray(inputs["n_lags"]))
    if not (2 <= n_lags <= 512):
        n_lags = 64
    alpha1 = float(np.asarray(inputs["a1"]).reshape(-1)[0])
    alpha2 = float(np.asarray(inputs["a2"]).reshape(-1)[0])
    NB = B // NCORES
    nc, C = _get(NB, n_lags, alpha1, alpha2)

    f32 = np.float32
    common = {
        "W1": np.ascontiguousarray(np.asarray(inputs["W1"], f32)),
        "W2": np.ascontiguousarray(np.asarray(inputs["W2"], f32)),
        "W3": np.ascontiguousarray(np.asarray(inputs["W3"], f32)),
        "Wo": np.ascontiguousarray(np.asarray(inputs["Wout"], f32)),
        "b1": np.asarray(inputs["b1"], f32).reshape(HID, 1),
        "b2": np.asarray(inputs["b2"], f32).reshape(HID, 1),
        "b3": np.asarray(inputs["b3"], f32).reshape(HID, 1),
        "TS": C["TS"], "ONES": C["ONES"],
        "PSEL": C["PSEL"], "SSEL": C["SSEL"],
    }
    in_maps = []
    for i in range(NCORES):
        m = dict(common)
        m["z"] = z[NB * i:NB * (i + 1)].reshape(NB, L * d)
        in_maps.append(m)

    res = run_bass_kernel_spmd(nc, in_maps, core_ids=list(range(NCORES)),
                               trace=trace)
    out = np.concatenate([r["out"].reshape(NB, n_lags, ODIM)
                          for r in res.results], axis=0)
    return out.astype(np.float32), res


def kernel(**inputs):
    out, _ = run(trace=False, **inputs)
    return out


# revision 12
# speedup vs baseline: 1.1452x; 1.1452x over previous
"""LogSigRNN generator kernel for Trainium2 (8 NeuronCores, data-parallel).

Self-contained: hardcodes problem shapes (B=4096, L=1000, d=3, hidden=256,
n_lags=64) and the sharding (batch / 8 cores). Computes the full forward:
Brownian path + depth-2 log-signature features + latched RNN, all on device.
"""
import numpy as np
from contextlib import ExitStack

import concourse.bass as bass
import concourse.tile as tile
from concourse import bacc, mybir
from concourse.bass_utils import run_bass_kernel_spmd
from concourse.masks import make_identity

F32 = mybir.dt.float32
F32R = mybir.dt.float32r
AF = mybir.ActivationFunctionType
ALU = mybir.AluOpType

LEN_NOISE = 1000
LEN_INTERVAL_U = 50
D = 3
HID = 256
LS = 6            # 3 + 3 logsig channels
ODIM = 3
NCORES = 8
PAIRS = [(0, 1), (0, 2), (1, 2)]


def _schedule(n_lags):
    """Interval indices + latch schedule (mirrors the model definition)."""
    tb = np.linspace(0.0, 1.0, LEN_NOISE)
    tu = tb[::LEN_INTERVAL_U]
    tt = np.linspace(0.0, 1.0, n_lags)
    ind_low, ind_max, u_list = [], [], []
    last_u = -1.0
    for t in tt[1:]:
        u = tu[tu < t].max()
        lo = int(np.nonzero(tb <= u)[0].max())
        if u != last_u:
            u_list.append(u)
            last_u = u
        hi = int(np.nonzero(tb <= t)[0].max())
        ind_low.append(lo)
        ind_max.append(hi)
    u_list.append(tt[-1])
    latch = np.zeros(n_lags, dtype=bool)
    q = list(u_list)
    for i, t in enumerate(tt):
        if q and t >= q[0]:
            q.pop(0)
            latch[i] = True
    return np.asarray(ind_low), np.asarray(ind_max), latch


def _constants(n_lags):
    """Host-side constant matrices fed to the device."""
    K = n_lags - 1
    ind_low, ind_max, latch = _schedule(n_lags)
    NZ = LEN_NOISE - 1                      # 999 usable z rows (1..999)
    NT = (NZ + 127) // 128                  # 8 L-tiles
    cnt = [min(128, NZ - 128 * j) for j in range(NT)]

    TS = np.zeros((128, 128), np.float32)   # strict lower triangle
    for k in range(128):
        TS[k, k + 1:] = 1.0
    ONES = np.ones((128, 128), np.float32)

    PSEL = np.zeros((128, NT * K), np.float32)
    SSEL = np.zeros((128, NT * K), np.float32)
    for j in range(NT):
        for p in range(cnt[j]):
            zrow = 1 + 128 * j + p          # z index for zs row p of tile j
            l = 128 * j + p                 # anti/bp index
            for k in range(K):
                if zrow <= ind_max[k]:
                    PSEL[p, K * j + k] = 1.0
                if ind_low[k] <= l < ind_max[k]:
                    SSEL[p, K * j + k] = 1.0

    return dict(TS=TS, ONES=ONES, PSEL=PSEL, SSEL=SSEL,
                latch=latch, cnt=cnt, NT=NT, K=K)


_CACHE = {}


def _build(NB, n_lags, alpha1, alpha2):
    """Build + finalize the bass module for one core's shard of size NB."""
    C = _constants(n_lags)
    NT, K, cnt = C["NT"], C["K"], C["cnt"]
    sqrt_dt = float(np.sqrt(1.0 / (LEN_NOISE - 1)))

    nc = bacc.Bacc("TRN2", target_bir_lowering=False, debug=False)

    d_z = nc.dram_tensor("z", [NB, LEN_NOISE * D], F32, kind="ExternalInput")
    d_W1 = nc.dram_tensor("W1", [HID + LS, HID], F32R, kind="ExternalInput")
    d_W2 = nc.dram_tensor("W2", [HID, HID], F32R, kind="ExternalInput")
    d_W3 = nc.dram_tensor("W3", [HID, HID], F32R, kind="ExternalInput")
    d_Wo = nc.dram_tensor("Wo", [HID, ODIM], F32R, kind="ExternalInput")
    d_b1 = nc.dram_tensor("b1", [HID, 1], F32, kind="ExternalInput")
    d_b2 = nc.dram_tensor("b2", [HID, 1], F32, kind="ExternalInput")
    d_b3 = nc.dram_tensor("b3", [HID, 1], F32, kind="ExternalInput")
    d_TS = nc.dram_tensor("TS", [128, 128], F32R, kind="ExternalInput")
    d_ONES = nc.dram_tensor("ONES", [128, 128], F32R, kind="ExternalInput")
    d_PSEL = nc.dram_tensor("PSEL", [128, NT * K], F32R, kind="ExternalInput")
    d_SSEL = nc.dram_tensor("SSEL", [128, NT * K], F32R, kind="ExternalInput")
    d_out = nc.dram_tensor("out", [NB, n_lags * ODIM], F32, kind="ExternalOutput")
    d_sig = nc.dram_tensor("sigsc", [LS, n_lags * NB], F32R)  # internal scratch

    NBT = NB // 128  # batch tiles
    NO2 = (n_lags + 1) // 2

    with tile.TileContext(nc) as tc, ExitStack() as ctx:
        cons = ctx.enter_context(tc.tile_pool(name="cons", bufs=1))
        outp = ctx.enter_context(tc.tile_pool(name="outp", bufs=1))

        # ---- constants / weights to SBUF ----
        sTS = cons.tile([128, 128], F32R); nc.sync.dma_start(sTS[:], d_TS[:])
        sONES = cons.tile([128, 128], F32R); nc.sync.dma_start(sONES[:], d_ONES[:])
        sPSEL = cons.tile([128, NT * K], F32R); nc.sync.dma_start(sPSEL[:], d_PSEL[:])
        sSSEL = cons.tile([128, NT * K], F32R); nc.sync.dma_start(sSSEL[:], d_SSEL[:])
        ident = cons.tile([128, 128], F32)
        make_identity(nc, ident[:])

        W1a = cons.tile([128, HID], F32R); nc.sync.dma_start(W1a[:], d_W1[0:128, :])
        W1b = cons.tile([128, HID], F32R); nc.sync.dma_start(W1b[:], d_W1[128:256, :])
        W1s = cons.tile([LS, HID], F32R); nc.sync.dma_start(W1s[:], d_W1[256:262, :])
        W2a = cons.tile([128, HID], F32R); nc.sync.dma_start(W2a[:], d_W2[0:128, :])
        W2b = cons.tile([128, HID], F32R); nc.sync.dma_start(W2b[:], d_W2[128:256, :])
        W3a = cons.tile([128, HID], F32R); nc.sync.dma_start(W3a[:], d_W3[0:128, :])
        W3b = cons.tile([128, HID], F32R); nc.sync.dma_start(W3b[:], d_W3[128:256, :])
        Woa = cons.tile([128, ODIM], F32R); nc.sync.dma_start(Woa[:], d_Wo[0:128, :])
        Wob = cons.tile([128, ODIM], F32R); nc.sync.dma_start(Wob[:], d_Wo[128:256, :])
        b1a = cons.tile([128, 1], F32); nc.sync.dma_start(b1a[:], d_b1[0:128, :])
        b1b = cons.tile([128, 1], F32); nc.sync.dma_start(b1b[:], d_b1[128:256, :])
        b2a = cons.tile([128, 1], F32); nc.sync.dma_start(b2a[:], d_b2[0:128, :])
        b2b = cons.tile([128, 1], F32); nc.sync.dma_start(b2b[:], d_b2[128:256, :])
        b3a = cons.tile([128, 1], F32); nc.sync.dma_start(b3a[:], d_b3[0:128, :])
        b3b = cons.tile([128, 1], F32); nc.sync.dma_start(b3b[:], d_b3[128:256, :])

        out_all = [outp.tile([ODIM * NO2, NB], F32, tag=f"oall{h}", name=f"oall{h}")
                   for h in range(2)]

        eng_alt = [nc.scalar, nc.vector]

        def copy_out(eng, dst, src):
            if eng is nc.scalar:
                eng.activation(dst, src, AF.Copy)
            else:
                eng.tensor_copy(dst, src)

        # =================== preprocessing ===================
        with tc.tile_pool(name="zs", bufs=1) as zsp, \
             tc.tile_pool(name="bp", bufs=1) as bpp, \
             tc.tile_pool(name="segsb", bufs=1) as segsb_p, \
             tc.tile_pool(name="znat", bufs=2) as znat_p, \
             tc.tile_pool(name="prod", bufs=6) as prod_p, \
             tc.tile_pool(name="tp_ps", bufs=4, space="PSUM") as tp_ps, \
             tc.tile_pool(name="bp_ps", bufs=2, space="PSUM") as bp_ps, \
             tc.tile_pool(name="seg_ps", bufs=2, space="PSUM") as seg_ps:

            zs = [[zsp.tile([128, NB], F32R, tag=f"zs{c}_{j}", name=f"zs{c}_{j}")
                   for j in range(NT)]
                  for c in range(D)]
            bp = [[bpp.tile([128, NB], F32R, tag=f"bp{c}_{j}", name=f"bp{c}_{j}")
                   for j in range(NT)]
                  for c in range(D)]


            # ---- load z natural + transpose to [L, batch] per channel ----
            for bt in range(NBT):
                znat = znat_p.tile([128, LEN_NOISE * D], F32)
                nc.sync.dma_start(znat[:], d_z[128 * bt:128 * (bt + 1), :])
                for c in range(D):
                    for j in range(NT):
                        ps = tp_ps.tile([128, 128], F32, tag="tp")
                        off = D * (1 + 128 * j) + c
                        nc.tensor.transpose(
                            ps[0:cnt[j], :],
                            znat[:, off:off + D * (cnt[j] - 1) + 1:D],
                            ident[:])
                        copy_out(eng_alt[(c + j) % 2],
                                 zs[c][j][0:cnt[j], 128 * bt:128 * (bt + 1)],
                                 ps[0:cnt[j], :])

            # ---- bp[j] = colsum(zs[<j]) + strict_cumsum(zs[j]) ----
            for c in range(D):
                for j in range(NT):
                    ps = bp_ps.tile([128, NB], F32, tag="bpps")
                    for jp in range(j):
                        nc.tensor.matmul(ps[:], sONES[0:cnt[jp], :],
                                         zs[c][jp][0:cnt[jp], :],
                                         start=(jp == 0), stop=False)
                    nc.tensor.matmul(ps[:], sTS[0:cnt[j], :], zs[c][j][0:cnt[j], :],
                                     start=(j == 0), stop=True)
                    copy_out(eng_alt[j % 2], bp[c][j][:], ps[:])

            # ---- level-1 + level-2 segment sums -> sig scratch in DRAM ----
            zf32 = cons.tile([128, NB], F32)
            nc.vector.memset(zf32[:], 0.0)
            zero6 = cons.tile([LS, NB], F32R)
            nc.scalar.activation(zero6[:], zf32[0:LS, :], AF.Copy)
            nc.sync.dma_start(d_sig[:, 0:NB], zero6[:])

            for c in range(D):
                ps = seg_ps.tile([K, NB], F32, tag="seg")
                for j in range(NT):
                    nc.tensor.matmul(ps[:], sPSEL[0:cnt[j], K * j:K * (j + 1)],
                                     zs[c][j][0:cnt[j], :],
                                     start=(j == 0), stop=(j == NT - 1))
                sb = segsb_p.tile([K, NB], F32R, tag=f"seg{c}")
                nc.scalar.activation(sb[:], ps[:], AF.Copy, scale=sqrt_dt)
                nc.sync.dma_start(
                    d_sig[c:c + 1, NB:].rearrange("s (k n) -> (s k) n", k=K), sb[:])

            for ip, (a, b) in enumerate(PAIRS):
                ps = seg_ps.tile([K, NB], F32, tag="seg")
                for j in range(NT):
                    cj = cnt[j]
                    q1 = prod_p.tile([128, NB], F32R, tag="q")
                    nc.vector.tensor_mul(q1[0:cj, :], bp[a][j][0:cj, :],
                                         zs[b][j][0:cj, :])
                    q2 = prod_p.tile([128, NB], F32R, tag="q")
                    nc.vector.tensor_mul(q2[0:cj, :], bp[b][j][0:cj, :],
                                         zs[a][j][0:cj, :])
                    anti = prod_p.tile([128, NB], F32R, tag="anti")
                    nc.vector.tensor_sub(anti[0:cj, :], q1[0:cj, :], q2[0:cj, :])
                    nc.tensor.matmul(ps[:], sSSEL[0:cnt[j], K * j:K * (j + 1)],
                                     anti[0:cnt[j], :],
                                     start=(j == 0), stop=(j == NT - 1))
                sb = segsb_p.tile([K, NB], F32R, tag=f"seg{3 + ip}")
                nc.scalar.activation(sb[:], ps[:], AF.Copy,
                                     scale=0.5 * sqrt_dt * sqrt_dt)
                nc.sync.dma_start(
                    d_sig[3 + ip:4 + ip, NB:].rearrange("s (k n) -> (s k) n", k=K),
                    sb[:])

        # =================== RNN ===================
        with tc.tile_pool(name="hst", bufs=1) as hst_p, \
             tc.tile_pool(name="sigt", bufs=6) as sigt_p, \
             tc.tile_pool(name="u", bufs=4) as u_p, \
             tc.tile_pool(name="h3", bufs=6) as h3_p, \
             tc.tile_pool(name="dvet", bufs=4) as dvet_p, \
             tc.tile_pool(name="osb", bufs=4) as osb_p, \
             tc.tile_pool(name="p1", bufs=3, space="PSUM") as p1_p, \
             tc.tile_pool(name="p2", bufs=2, space="PSUM") as p2_p, \
             tc.tile_pool(name="p3", bufs=2, space="PSUM") as p3_p, \
             tc.tile_pool(name="po", bufs=1, space="PSUM") as po_p:

            zf32b = hst_p.tile([128, NB], F32, tag="zf32b")
            nc.vector.memset(zf32b[:], 0.0)
            h0a = hst_p.tile([128, NB], F32R, tag="h0a")
            h0b = hst_p.tile([128, NB], F32R, tag="h0b")
            nc.scalar.activation(h0a[:], zf32b[:], AF.Copy)
            nc.scalar.activation(h0b[:], zf32b[:], AF.Copy)
            state = (h0a, h0b)
            latch = C["latch"]

            def prelu_act(dst, psrc, bias_t, alpha):
                nc.scalar.activation(dst, psrc, AF.Prelu, bias=bias_t, alpha=alpha)

            def prelu_dve(dst, psrc, bias_t, alpha):
                t1 = dvet_p.tile([128, NB], F32, tag="t1")
                nc.vector.tensor_scalar_add(t1[:], psrc, bias_t)
                nc.vector.scalar_tensor_tensor(dst, t1[:], alpha, t1[:],
                                               op0=ALU.mult, op1=ALU.max)

            # Emission order: within each latch group, emit the latch step
            # (the group's last step) FIRST so its tanh->L1 critical chain
            # starts immediately; the group's leaf steps fill PE bubbles.
            groups, cur = [], []
            for t in range(n_lags):
                cur.append(t)
                if latch[t]:
                    groups.append(cur)
                    cur = []
            if cur:
                groups.append(cur)
            ordered = []
            for grp in groups:
                if latch[grp[-1]]:
                    ordered += [grp[-1]] + grp[:-1]
                else:
                    ordered += grp

            h3_latch = {}
            for t in ordered:
                ha, hb = state
                sigt = sigt_p.tile([LS, NB], F32R, tag="sigt")
                nc.sync.dma_start(sigt[:], d_sig[:, t * NB:(t + 1) * NB])

                u1 = []
                for i in range(2):
                    ps = p1_p.tile([128, NB], F32, tag="p1")
                    cs = slice(128 * i, 128 * (i + 1))
                    nc.tensor.matmul(ps[:], W1a[:, cs], ha[:], start=True, stop=False)
                    nc.tensor.matmul(ps[:], W1b[:, cs], hb[:], start=False, stop=False)
                    nc.tensor.matmul(ps[:], W1s[:, cs], sigt[:], start=False, stop=True)
                    u = u_p.tile([128, NB], F32R, tag="u1")
                    if i == 0:
                        prelu_act(u[:], ps[:], b1a[:, 0:1], alpha1)
                    else:
                        prelu_dve(u[:], ps[:], b1b[:, 0:1], alpha1)
                    u1.append(u)

                u2 = []
                for i in range(2):
                    ps = p2_p.tile([128, NB], F32, tag="p2")
                    cs = slice(128 * i, 128 * (i + 1))
                    nc.tensor.matmul(ps[:], W2a[:, cs], u1[0][:], start=True, stop=False)
                    nc.tensor.matmul(ps[:], W2b[:, cs], u1[1][:], start=False, stop=True)
                    u = u_p.tile([128, NB], F32R, tag="u2")
                    if i == 0:
                        prelu_act(u[:], ps[:], b2a[:, 0:1], alpha2)
                    else:
                        prelu_dve(u[:], ps[:], b2b[:, 0:1], alpha2)
                    u2.append(u)

                h3 = []
                for i in range(2):
                    ps = p3_p.tile([128, NB], F32, tag="p3")
                    cs = slice(128 * i, 128 * (i + 1))
                    nc.tensor.matmul(ps[:], W3a[:, cs], u2[0][:], start=True, stop=False)
                    nc.tensor.matmul(ps[:], W3b[:, cs], u2[1][:], start=False, stop=True)
                    h = h3_p.tile([128, NB], F32R, tag="h3")
                    nc.scalar.activation(h[:], ps[:], AF.Tanh,
                                         bias=(b3a if i == 0 else b3b)[:, 0:1])
                    h3.append(h)

                ps = po_p.tile([ODIM, NB], F32, tag="po")
                nc.tensor.matmul(ps[:], Woa[:], h3[0][:], start=True, stop=False)
                nc.tensor.matmul(ps[:], Wob[:], h3[1][:], start=False, stop=True)
                osb = osb_p.tile([ODIM, NB], F32, tag="osb")
                nc.scalar.activation(osb[:], ps[:], AF.Copy)
                half, tm = divmod(t, NO2)
                nc.sync.dma_start(out_all[half][ODIM * tm:ODIM * (tm + 1), :],
                                  osb[:])

                if latch[t]:
                    h3_latch[t] = (h3[0], h3[1])
                gi = next(i for i, g in enumerate(groups) if t in g)
                if t == (groups[gi][-2] if len(groups[gi]) > 1 and
                         latch[groups[gi][-1]] else groups[gi][-1]):
                    lt = groups[gi][-1]
                    if latch[lt]:
                        state = h3_latch[lt]

        # ---- transpose outputs to batch-major and store ----
        with tc.tile_pool(name="ot_ps", bufs=2, space="PSUM") as ot_ps, \
             tc.tile_pool(name="otp", bufs=2) as ot_p:
            nrow = ODIM * NO2
            for bt in range(NBT):
                oT = ot_p.tile([128, ODIM * n_lags], F32, tag="oT")
                for half in range(2):
                    ps = ot_ps.tile([128, nrow], F32, tag="ot")
                    nc.tensor.transpose(ps[:, 0:nrow],
                                        out_all[half][0:nrow,
                                                      128 * bt:128 * (bt + 1)],
                                        ident[0:nrow, 0:nrow])
                    nc.scalar.activation(
                        oT[:, half * nrow:(half + 1) * nrow], ps[:, 0:nrow], AF.Copy)
                nc.sync.dma_start(d_out[128 * bt:128 * (bt + 1), :], oT[:])

    nc.finalize()
    return nc, C


def _get(NB, n_lags, alpha1, alpha2):
    key = (NB, n_lags, alpha1, alpha2)
    if key not in _CACHE:
        _CACHE[key] = _build(NB, n_lags, alpha1, alpha2)
    return _CACHE[key]


def run(trace=False, **inputs):
    z = np.ascontiguousarray(np.asarray(inputs["z"], dtype=np.float32))
    B, L, d = z.shape
    n_lags = int(np.asarray(inputs["n_lags"]))
    if not (2 <= n_lags <= 512):
        n_lags = 64
    alpha1 = float(np.asarray(inputs["a1"]).reshape(-1)[0])
    alpha2 = float(np.asarray(inputs["a2"]).reshape(-1)[0])
    NB = B // NCORES
    nc, C = _get(NB, n_lags, alpha1, alpha2)

    f32 = np.float32
    common = {
        "W1": np.ascontiguousarray(np.asarray(inputs["W1"], f32)),
        "W2": np.ascontiguousarray(np.asarray(inputs["W2"], f32)),
        "W3": np.ascontiguousarray(np.asarray(inputs["W3"], f32)),
        "Wo": np.ascontiguousarray(np.asarray(inputs["Wout"], f32)),
        "b1": np.asarray(inputs["b1"], f32).reshape(HID, 1),
        "b2": np.asarray(inputs["b2"], f32).reshape(HID, 1),
        "b3": np.asarray(inputs["b3"], f32).reshape(HID, 1),
        "TS": C["TS"], "ONES": C["ONES"],
        "PSEL": C["PSEL"], "SSEL": C["SSEL"],
    }
    in_maps = []
    for i in range(NCORES):
        m = dict(common)
        m["z"] = z[NB * i:NB * (i + 1)].reshape(NB, L * d)
        in_maps.append(m)

    res = run_bass_kernel_spmd(nc, in_maps, core_ids=list(range(NCORES)),
                               trace=trace)
    out = np.concatenate([r["out"].reshape(NB, n_lags, ODIM)
                          for r in res.results], axis=0)
    return out.astype(np.float32), res


def kernel(**inputs):
    out, _ = run(trace=False, **inputs)
    return out


# revision 16
# speedup vs baseline: 1.1519x; 1.0058x over previous
"""LogSigRNN generator kernel for Trainium2 (8 NeuronCores, data-parallel).

Self-contained: hardcodes problem shapes (B=4096, L=1000, d=3, hidden=256,
n_lags=64) and the sharding (batch / 8 cores). Computes the full forward:
Brownian path + depth-2 log-signature features + latched RNN, all on device.
"""
import numpy as np
from contextlib import ExitStack

import concourse.bass as bass
import concourse.tile as tile
from concourse import bacc, mybir
from concourse.bass_utils import run_bass_kernel_spmd
from concourse.masks import make_identity

F32 = mybir.dt.float32
F32R = mybir.dt.float32r
AF = mybir.ActivationFunctionType
ALU = mybir.AluOpType

LEN_NOISE = 1000
LEN_INTERVAL_U = 50
D = 3
HID = 256
LS = 6            # 3 + 3 logsig channels
ODIM = 3
NCORES = 8
PAIRS = [(0, 1), (0, 2), (1, 2)]


def _schedule(n_lags):
    """Interval indices + latch schedule (mirrors the model definition)."""
    tb = np.linspace(0.0, 1.0, LEN_NOISE)
    tu = tb[::LEN_INTERVAL_U]
    tt = np.linspace(0.0, 1.0, n_lags)
    ind_low, ind_max, u_list = [], [], []
    last_u = -1.0
    for t in tt[1:]:
        u = tu[tu < t].max()
        lo = int(np.nonzero(tb <= u)[0].max())
        if u != last_u:
            u_list.append(u)
            last_u = u
        hi = int(np.nonzero(tb <= t)[0].max())
        ind_low.append(lo)
        ind_max.append(hi)
    u_list.append(tt[-1])
    latch = np.zeros(n_lags, dtype=bool)
    q = list(u_list)
    for i, t in enumerate(tt):
        if q and t >= q[0]:
            q.pop(0)
            latch[i] = True
    return np.asarray(ind_low), np.asarray(ind_max), latch


def _constants(n_lags):
    """Host-side constant matrices fed to the device."""
    K = n_lags - 1
    ind_low, ind_max, latch = _schedule(n_lags)
    NZ = LEN_NOISE - 1                      # 999 usable z rows (1..999)
    NT = (NZ + 127) // 128                  # 8 L-tiles
    cnt = [min(128, NZ - 128 * j) for j in range(NT)]

    TS = np.zeros((128, 128), np.float32)   # strict lower triangle
    for k in range(128):
        TS[k, k + 1:] = 1.0
    ONES = np.ones((128, 128), np.float32)

    PSEL = np.zeros((128, NT * K), np.float32)
    SSEL = np.zeros((128, NT * K), np.float32)
    for j in range(NT):
        for p in range(cnt[j]):
            zrow = 1 + 128 * j + p          # z index for zs row p of tile j
            l = 128 * j + p                 # anti/bp index
            for k in range(K):
                if zrow <= ind_max[k]:
                    PSEL[p, K * j + k] = 1.0
                if ind_low[k] <= l < ind_max[k]:
                    SSEL[p, K * j + k] = 1.0

    return dict(TS=TS, ONES=ONES, PSEL=PSEL, SSEL=SSEL,
                latch=latch, cnt=cnt, NT=NT, K=K)


_CACHE = {}


def _build(NB, n_lags, alpha1, alpha2):
    """Build + finalize the bass module for one core's shard of size NB."""
    C = _constants(n_lags)
    NT, K, cnt = C["NT"], C["K"], C["cnt"]
    sqrt_dt = float(np.sqrt(1.0 / (LEN_NOISE - 1)))

    nc = bacc.Bacc("TRN2", target_bir_lowering=False, debug=False)

    d_z = nc.dram_tensor("z", [NB, LEN_NOISE * D], F32, kind="ExternalInput")
    d_W1 = nc.dram_tensor("W1", [HID + LS, HID], F32R, kind="ExternalInput")
    d_W2 = nc.dram_tensor("W2", [HID, HID], F32R, kind="ExternalInput")
    d_W3 = nc.dram_tensor("W3", [HID, HID], F32R, kind="ExternalInput")
    d_Wo = nc.dram_tensor("Wo", [HID, ODIM], F32R, kind="ExternalInput")
    d_b1 = nc.dram_tensor("b1", [HID, 1], F32, kind="ExternalInput")
    d_b2 = nc.dram_tensor("b2", [HID, 1], F32, kind="ExternalInput")
    d_b3 = nc.dram_tensor("b3", [HID, 1], F32, kind="ExternalInput")
    d_TS = nc.dram_tensor("TS", [128, 128], F32R, kind="ExternalInput")
    d_ONES = nc.dram_tensor("ONES", [128, 128], F32R, kind="ExternalInput")
    d_PSEL = nc.dram_tensor("PSEL", [128, NT * K], F32R, kind="ExternalInput")
    d_SSEL = nc.dram_tensor("SSEL", [128, NT * K], F32R, kind="ExternalInput")
    d_out = nc.dram_tensor("out", [NB, n_lags * ODIM], F32, kind="ExternalOutput")
    d_sig = nc.dram_tensor("sigsc", [LS, n_lags * NB], F32R)  # internal scratch

    NBT = NB // 128  # batch tiles
    NO2 = (n_lags + 1) // 2

    with tile.TileContext(nc) as tc, ExitStack() as ctx:
        cons = ctx.enter_context(tc.tile_pool(name="cons", bufs=1))
        outp = ctx.enter_context(tc.tile_pool(name="outp", bufs=1))

        # ---- constants / weights to SBUF ----
        sTS = cons.tile([128, 128], F32R); nc.sync.dma_start(sTS[:], d_TS[:])
        sONES = cons.tile([128, 128], F32R); nc.sync.dma_start(sONES[:], d_ONES[:])
        sPSEL = cons.tile([128, NT * K], F32R); nc.sync.dma_start(sPSEL[:], d_PSEL[:])
        sSSEL = cons.tile([128, NT * K], F32R); nc.sync.dma_start(sSSEL[:], d_SSEL[:])
        ident = cons.tile([128, 128], F32)
        make_identity(nc, ident[:])

        W1a = cons.tile([128, HID], F32R); nc.sync.dma_start(W1a[:], d_W1[0:128, :])
        W1b = cons.tile([128, HID], F32R); nc.sync.dma_start(W1b[:], d_W1[128:256, :])
        W1s = cons.tile([LS, HID], F32R); nc.sync.dma_start(W1s[:], d_W1[256:262, :])
        W2a = cons.tile([128, HID], F32R); nc.sync.dma_start(W2a[:], d_W2[0:128, :])
        W2b = cons.tile([128, HID], F32R); nc.sync.dma_start(W2b[:], d_W2[128:256, :])
        W3a = cons.tile([128, HID], F32R); nc.sync.dma_start(W3a[:], d_W3[0:128, :])
        W3b = cons.tile([128, HID], F32R); nc.sync.dma_start(W3b[:], d_W3[128:256, :])
        Woa = cons.tile([128, ODIM], F32R); nc.sync.dma_start(Woa[:], d_Wo[0:128, :])
        Wob = cons.tile([128, ODIM], F32R); nc.sync.dma_start(Wob[:], d_Wo[128:256, :])
        b1a = cons.tile([128, 1], F32); nc.sync.dma_start(b1a[:], d_b1[0:128, :])
        b1b = cons.tile([128, 1], F32); nc.sync.dma_start(b1b[:], d_b1[128:256, :])
        b2a = cons.tile([128, 1], F32); nc.sync.dma_start(b2a[:], d_b2[0:128, :])
        b2b = cons.tile([128, 1], F32); nc.sync.dma_start(b2b[:], d_b2[128:256, :])
        b3a = cons.tile([128, 1], F32); nc.sync.dma_start(b3a[:], d_b3[0:128, :])
        b3b = cons.tile([128, 1], F32); nc.sync.dma_start(b3b[:], d_b3[128:256, :])

        out_all = [outp.tile([ODIM * NO2, NB], F32, tag=f"oall{h}", name=f"oall{h}")
                   for h in range(2)]

        eng_alt = [nc.scalar, nc.vector]

        def copy_out(eng, dst, src):
            if eng is nc.scalar:
                eng.activation(dst, src, AF.Copy)
            else:
                eng.tensor_copy(dst, src)

        # =================== preprocessing ===================
        with tc.tile_pool(name="zs", bufs=1) as zsp, \
             tc.tile_pool(name="bp", bufs=1) as bpp, \
             tc.tile_pool(name="segsb", bufs=1) as segsb_p, \
             tc.tile_pool(name="znat", bufs=4) as znat_p, \
             tc.tile_pool(name="prod", bufs=6) as prod_p, \
             tc.tile_pool(name="tp_ps", bufs=4, space="PSUM") as tp_ps, \
             tc.tile_pool(name="bp_ps", bufs=2, space="PSUM") as bp_ps, \
             tc.tile_pool(name="seg_ps", bufs=2, space="PSUM") as seg_ps:

            zs = [[zsp.tile([128, NB], F32R, tag=f"zs{c}_{j}", name=f"zs{c}_{j}")
                   for j in range(NT)]
                  for c in range(D)]
            bp = [[bpp.tile([128, NB], F32R, tag=f"bp{c}_{j}", name=f"bp{c}_{j}")
                   for j in range(NT)]
                  for c in range(D)]


            # ---- load z natural + transpose to [L, batch] per channel ----
            for bt in range(NBT):
                znat = znat_p.tile([128, LEN_NOISE * D], F32)
                nc.sync.dma_start(znat[:], d_z[128 * bt:128 * (bt + 1), :])
                for c in range(D):
                    for j in range(NT):
                        ps = tp_ps.tile([128, 128], F32, tag="tp")
                        off = D * (1 + 128 * j) + c
                        nc.tensor.transpose(
                            ps[0:cnt[j], :],
                            znat[:, off:off + D * (cnt[j] - 1) + 1:D],
                            ident[:])
                        copy_out(eng_alt[(c + j) % 2],
                                 zs[c][j][0:cnt[j], 128 * bt:128 * (bt + 1)],
                                 ps[0:cnt[j], :])

            # ---- bp[j] = colsum(zs[<j]) + strict_cumsum(zs[j]) ----
            for c in range(D):
                for j in range(NT):
                    ps = bp_ps.tile([128, NB], F32, tag="bpps")
                    for jp in range(j):
                        nc.tensor.matmul(ps[:], sONES[0:cnt[jp], :],
                                         zs[c][jp][0:cnt[jp], :],
                                         start=(jp == 0), stop=False)
                    nc.tensor.matmul(ps[:], sTS[0:cnt[j], :], zs[c][j][0:cnt[j], :],
                                     start=(j == 0), stop=True)
                    copy_out(eng_alt[j % 2], bp[c][j][:], ps[:])

            # ---- level-1 + level-2 segment sums -> sig scratch in DRAM ----
            zf32 = cons.tile([128, NB], F32)
            nc.vector.memset(zf32[:], 0.0)
            zero6 = cons.tile([LS, NB], F32R)
            nc.scalar.activation(zero6[:], zf32[0:LS, :], AF.Copy)
            nc.sync.dma_start(d_sig[:, 0:NB], zero6[:])

            for c in range(D):
                ps = seg_ps.tile([K, NB], F32, tag="seg")
                for j in range(NT):
                    nc.tensor.matmul(ps[:], sPSEL[0:cnt[j], K * j:K * (j + 1)],
                                     zs[c][j][0:cnt[j], :],
                                     start=(j == 0), stop=(j == NT - 1))
                sb = segsb_p.tile([K, NB], F32R, tag=f"seg{c}")
                nc.scalar.activation(sb[:], ps[:], AF.Copy, scale=sqrt_dt)
                nc.sync.dma_start(
                    d_sig[c:c + 1, NB:].rearrange("s (k n) -> (s k) n", k=K), sb[:])

            for ip, (a, b) in enumerate(PAIRS):
                ps = seg_ps.tile([K, NB], F32, tag="seg")
                for j in range(NT):
                    cj = cnt[j]
                    q1 = prod_p.tile([128, NB], F32R, tag="q")
                    nc.vector.tensor_mul(q1[0:cj, :], bp[a][j][0:cj, :],
                                         zs[b][j][0:cj, :])
                    q2 = prod_p.tile([128, NB], F32R, tag="q")
                    nc.vector.tensor_mul(q2[0:cj, :], bp[b][j][0:cj, :],
                                         zs[a][j][0:cj, :])
                    anti = prod_p.tile([128, NB], F32R, tag="anti")
                    nc.vector.tensor_sub(anti[0:cj, :], q1[0:cj, :], q2[0:cj, :])
                    nc.tensor.matmul(ps[:], sSSEL[0:cnt[j], K * j:K * (j + 1)],
                                     anti[0:cnt[j], :],
                                     start=(j == 0), stop=(j == NT - 1))
                sb = segsb_p.tile([K, NB], F32R, tag=f"seg{3 + ip}")
                nc.scalar.activation(sb[:], ps[:], AF.Copy,
                                     scale=0.5 * sqrt_dt * sqrt_dt)
                nc.sync.dma_start(
                    d_sig[3 + ip:4 + ip, NB:].rearrange("s (k n) -> (s k) n", k=K),
                    sb[:])

        # =================== RNN ===================
        with tc.tile_pool(name="hst", bufs=1) as hst_p, \
             tc.tile_pool(name="sigt", bufs=6) as sigt_p, \
             tc.tile_pool(name="u", bufs=4) as u_p, \
             tc.tile_pool(name="h3", bufs=6) as h3_p, \
             tc.tile_pool(name="dvet", bufs=4) as dvet_p, \
             tc.tile_pool(name="osb", bufs=4) as osb_p, \
             tc.tile_pool(name="p1", bufs=3, space="PSUM") as p1_p, \
             tc.tile_pool(name="p2", bufs=2, space="PSUM") as p2_p, \
             tc.tile_pool(name="p3", bufs=2, space="PSUM") as p3_p, \
             tc.tile_pool(name="po", bufs=1, space="PSUM") as po_p:

            zf32b = hst_p.tile([128, NB], F32, tag="zf32b")
            nc.vector.memset(zf32b[:], 0.0)
            h0a = hst_p.tile([128, NB], F32R, tag="h0a")
            h0b = hst_p.tile([128, NB], F32R, tag="h0b")
            nc.scalar.activation(h0a[:], zf32b[:], AF.Copy)
            nc.scalar.activation(h0b[:], zf32b[:], AF.Copy)
            state = (h0a, h0b)
            latch = C["latch"]

            def prelu_act(dst, psrc, bias_t, alpha):
                nc.scalar.activation(dst, psrc, AF.Prelu, bias=bias_t, alpha=alpha)

            def prelu_dve(dst, psrc, bias_t, alpha):
                t1 = dvet_p.tile([128, NB], F32, tag="t1")
                nc.vector.tensor_scalar_add(t1[:], psrc, bias_t)
                nc.vector.scalar_tensor_tensor(dst, t1[:], alpha, t1[:],
                                               op0=ALU.mult, op1=ALU.max)

            # Emission order: within each latch group, emit the latch step
            # (the group's last step) FIRST so its tanh->L1 critical chain
            # starts immediately; the group's leaf steps fill PE bubbles.
            groups, cur = [], []
            for t in range(n_lags):
                cur.append(t)
                if latch[t]:
                    groups.append(cur)
                    cur = []
            if cur:
                groups.append(cur)
            ordered = []
            for grp in groups:
                if latch[grp[-1]]:
                    ordered += [grp[-1]] + grp[:-1]
                else:
                    ordered += grp

            h3_latch = {}
            for t in ordered:
                ha, hb = state
                sigt = sigt_p.tile([LS, NB], F32R, tag="sigt")
                nc.sync.dma_start(sigt[:], d_sig[:, t * NB:(t + 1) * NB])

                u1 = []
                for i in range(2):
                    ps = p1_p.tile([128, NB], F32, tag="p1")
                    cs = slice(128 * i, 128 * (i + 1))
                    nc.tensor.matmul(ps[:], W1a[:, cs], ha[:], start=True, stop=False)
                    nc.tensor.matmul(ps[:], W1b[:, cs], hb[:], start=False, stop=False)
                    nc.tensor.matmul(ps[:], W1s[:, cs], sigt[:], start=False, stop=True)
                    u = u_p.tile([128, NB], F32R, tag="u1")
                    if i == 0:
                        prelu_act(u[:], ps[:], b1a[:, 0:1], alpha1)
                    else:
                        prelu_dve(u[:], ps[:], b1b[:, 0:1], alpha1)
                    u1.append(u)

                u2 = []
                for i in range(2):
                    ps = p2_p.tile([128, NB], F32, tag="p2")
                    cs = slice(128 * i, 128 * (i + 1))
                    nc.tensor.matmul(ps[:], W2a[:, cs], u1[0][:], start=True, stop=False)
                    nc.tensor.matmul(ps[:], W2b[:, cs], u1[1][:], start=False, stop=True)
                    u = u_p.tile([128, NB], F32R, tag="u2")
                    if i == 0:
                        prelu_act(u[:], ps[:], b2a[:, 0:1], alpha2)
                    else:
                        prelu_dve(u[:], ps[:], b2b[:, 0:1], alpha2)
                    u2.append(u)

                h3 = []
                for i in range(2):
                    ps = p3_p.tile([128, NB], F32, tag="p3")
                    cs = slice(128 * i, 128 * (i + 1))
                    nc.tensor.matmul(ps[:], W3a[:, cs], u2[0][:], start=True, stop=False)
                    nc.tensor.matmul(ps[:], W3b[:, cs], u2[1][:], start=False, stop=True)
                    h = h3_p.tile([128, NB], F32R, tag="h3")
                    nc.scalar.activation(h[:], ps[:], AF.Tanh,
                                         bias=(b3a if i == 0 else b3b)[:, 0:1])
                    h3.append(h)

                ps = po_p.tile([ODIM, NB], F32, tag="po")
                nc.tensor.matmul(ps[:], Woa[:], h3[0][:], start=True, stop=False)
                nc.tensor.matmul(ps[:], Wob[:], h3[1][:], start=False, stop=True)
                osb = osb_p.tile([ODIM, NB], F32, tag="osb")
                nc.scalar.activation(osb[:], ps[:], AF.Copy)
                half, tm = divmod(t, NO2)
                nc.sync.dma_start(out_all[half][ODIM * tm:ODIM * (tm + 1), :],
                                  osb[:])

                if latch[t]:
                    h3_latch[t] = (h3[0], h3[1])
                gi = next(i for i, g in enumerate(groups) if t in g)
                if t == (groups[gi][-2] if len(groups[gi]) > 1 and
                         latch[groups[gi][-1]] else groups[gi][-1]):
                    lt = groups[gi][-1]
                    if latch[lt]:
                        state = h3_latch[lt]

        # ---- transpose outputs to batch-major and store ----
        with tc.tile_pool(name="ot_ps", bufs=2, space="PSUM") as ot_ps, \
             tc.tile_pool(name="otp", bufs=2) as ot_p:
            nrow = ODIM * NO2
            for bt in range(NBT):
                oT = ot_p.tile([128, ODIM * n_lags], F32, tag="oT")
                for half in range(2):
                    ps = ot_ps.tile([128, nrow], F32, tag="ot")
                    nc.tensor.transpose(ps[:, 0:nrow],
                                        out_all[half][0:nrow,
                                                      128 * bt:128 * (bt + 1)],
                                        ident[0:nrow, 0:nrow])
                    nc.scalar.activation(
                        oT[:, half * nrow:(half + 1) * nrow], ps[:, 0:nrow], AF.Copy)
                nc.sync.dma_start(d_out[128 * bt:128 * (bt + 1), :], oT[:])

    nc.finalize()
    return nc, C


def _get(NB, n_lags, alpha1, alpha2):
    key = (NB, n_lags, alpha1, alpha2)
    if key not in _CACHE:
        _CACHE[key] = _build(NB, n_lags, alpha1, alpha2)
    return _CACHE[key]


def run(trace=False, **inputs):
    z = np.ascontiguousarray(np.asarray(inputs["z"], dtype=np.float32))
    B, L, d = z.shape
    n_lags = int(np.asarray(inputs["n_lags"]))
    if not (2 <= n_lags <= 512):
        n_lags = 64
    alpha1 = float(np.asarray(inputs["a1"]).reshape(-1)[0])
    alpha2 = float(np.asarray(inputs["a2"]).reshape(-1)[0])
    NB = B // NCORES
    nc, C = _get(NB, n_lags, alpha1, alpha2)

    f32 = np.float32
    common = {
        "W1": np.ascontiguousarray(np.asarray(inputs["W1"], f32)),
        "W2": np.ascontiguousarray(np.asarray(inputs["W2"], f32)),
        "W3": np.ascontiguousarray(np.asarray(inputs["W3"], f32)),
        "Wo": np.ascontiguousarray(np.asarray(inputs["Wout"], f32)),
        "b1": np.asarray(inputs["b1"], f32).reshape(HID, 1),
        "b2": np.asarray(inputs["b2"], f32).reshape(HID, 1),
        "b3": np.asarray(inputs["b3"], f32).reshape(HID, 1),
        "TS": C["TS"], "ONES": C["ONES"],
        "PSEL": C["PSEL"], "SSEL": C["SSEL"],
    }
    in_maps = []
    for i in range(NCORES):
        m = dict(common)
        m["z"] = z[NB * i:NB * (i + 1)].reshape(NB, L * d)
        in_maps.append(m)

    res = run_bass_kernel_spmd(nc, in_maps, core_ids=list(range(NCORES)),
                               trace=trace)
    out = np.concatenate([r["out"].reshape(NB, n_lags, ODIM)
                          for r in res.results], axis=0)
    return out.astype(np.float32), res


def kernel(**inputs):
    out, _ = run(trace=False, **inputs)
    return out


# revision 18
# speedup vs baseline: 1.1532x; 1.0012x over previous
"""LogSigRNN generator kernel for Trainium2 (8 NeuronCores, data-parallel).

Self-contained: hardcodes problem shapes (B=4096, L=1000, d=3, hidden=256,
n_lags=64) and the sharding (batch / 8 cores). Computes the full forward:
Brownian path + depth-2 log-signature features + latched RNN, all on device.
"""
import numpy as np
from contextlib import ExitStack

import concourse.bass as bass
import concourse.tile as tile
from concourse import bacc, mybir
from concourse.bass_utils import run_bass_kernel_spmd
from concourse.masks import make_identity

F32 = mybir.dt.float32
F32R = mybir.dt.float32r
AF = mybir.ActivationFunctionType
ALU = mybir.AluOpType

LEN_NOISE = 1000
LEN_INTERVAL_U = 50
D = 3
HID = 256
LS = 6            # 3 + 3 logsig channels
ODIM = 3
NCORES = 8
PAIRS = [(0, 1), (0, 2), (1, 2)]


def _schedule(n_lags):
    """Interval indices + latch schedule (mirrors the model definition)."""
    tb = np.linspace(0.0, 1.0, LEN_NOISE)
    tu = tb[::LEN_INTERVAL_U]
    tt = np.linspace(0.0, 1.0, n_lags)
    ind_low, ind_max, u_list = [], [], []
    last_u = -1.0
    for t in tt[1:]:
        u = tu[tu < t].max()
        lo = int(np.nonzero(tb <= u)[0].max())
        if u != last_u:
            u_list.append(u)
            last_u = u
        hi = int(np.nonzero(tb <= t)[0].max())
        ind_low.append(lo)
        ind_max.append(hi)
    u_list.append(tt[-1])
    latch = np.zeros(n_lags, dtype=bool)
    q = list(u_list)
    for i, t in enumerate(tt):
        if q and t >= q[0]:
            q.pop(0)
            latch[i] = True
    return np.asarray(ind_low), np.asarray(ind_max), latch


def _constants(n_lags):
    """Host-side constant matrices fed to the device."""
    K = n_lags - 1
    ind_low, ind_max, latch = _schedule(n_lags)
    NZ = LEN_NOISE - 1                      # 999 usable z rows (1..999)
    NT = (NZ + 127) // 128                  # 8 L-tiles
    cnt = [min(128, NZ - 128 * j) for j in range(NT)]

    TS = np.zeros((128, 128), np.float32)   # strict lower triangle
    for k in range(128):
        TS[k, k + 1:] = 1.0
    ONES = np.ones((128, 128), np.float32)

    PSEL = np.zeros((128, NT * K), np.float32)
    SSEL = np.zeros((128, NT * K), np.float32)
    for j in range(NT):
        for p in range(cnt[j]):
            zrow = 1 + 128 * j + p          # z index for zs row p of tile j
            l = 128 * j + p                 # anti/bp index
            for k in range(K):
                if zrow <= ind_max[k]:
                    PSEL[p, K * j + k] = 1.0
                if ind_low[k] <= l < ind_max[k]:
                    SSEL[p, K * j + k] = 1.0

    return dict(TS=TS, ONES=ONES, PSEL=PSEL, SSEL=SSEL,
                latch=latch, cnt=cnt, NT=NT, K=K)


_CACHE = {}


def _build(NB, n_lags, alpha1, alpha2):
    """Build + finalize the bass module for one core's shard of size NB."""
    C = _constants(n_lags)
    NT, K, cnt = C["NT"], C["K"], C["cnt"]
    sqrt_dt = float(np.sqrt(1.0 / (LEN_NOISE - 1)))

    nc = bacc.Bacc("TRN2", target_bir_lowering=False, debug=False)

    d_z = nc.dram_tensor("z", [NB, LEN_NOISE * D], F32, kind="ExternalInput")
    d_W1 = nc.dram_tensor("W1", [HID + LS, HID], F32R, kind="ExternalInput")
    d_W2 = nc.dram_tensor("W2", [HID, HID], F32R, kind="ExternalInput")
    d_W3 = nc.dram_tensor("W3", [HID, HID], F32R, kind="ExternalInput")
    d_Wo = nc.dram_tensor("Wo", [HID, ODIM], F32R, kind="ExternalInput")
    d_b1 = nc.dram_tensor("b1", [HID, 1], F32, kind="ExternalInput")
    d_b2 = nc.dram_tensor("b2", [HID, 1], F32, kind="ExternalInput")
    d_b3 = nc.dram_tensor("b3", [HID, 1], F32, kind="ExternalInput")
    d_TS = nc.dram_tensor("TS", [128, 128], F32R, kind="ExternalInput")
    d_ONES = nc.dram_tensor("ONES", [128, 128], F32R, kind="ExternalInput")
    d_PSEL = nc.dram_tensor("PSEL", [128, NT * K], F32R, kind="ExternalInput")
    d_SSEL = nc.dram_tensor("SSEL", [128, NT * K], F32R, kind="ExternalInput")
    d_out = nc.dram_tensor("out", [NB, n_lags * ODIM], F32, kind="ExternalOutput")
    d_sig = nc.dram_tensor("sigsc", [LS, n_lags * NB], F32R)  # internal scratch

    NBT = NB // 128  # batch tiles
    NO2 = (n_lags + 1) // 2

    with tile.TileContext(nc) as tc, ExitStack() as ctx:
        cons = ctx.enter_context(tc.tile_pool(name="cons", bufs=1))
        outp = ctx.enter_context(tc.tile_pool(name="outp", bufs=1))

        # ---- constants / weights to SBUF ----
        sTS = cons.tile([128, 128], F32R); nc.sync.dma_start(sTS[:], d_TS[:])
        sONES = cons.tile([128, 128], F32R); nc.sync.dma_start(sONES[:], d_ONES[:])
        sPSEL = cons.tile([128, NT * K], F32R); nc.sync.dma_start(sPSEL[:], d_PSEL[:])
        sSSEL = cons.tile([128, NT * K], F32R); nc.sync.dma_start(sSSEL[:], d_SSEL[:])
        ident = cons.tile([128, 128], F32)
        make_identity(nc, ident[:])

        W1a = cons.tile([128, HID], F32R); nc.sync.dma_start(W1a[:], d_W1[0:128, :])
        W1b = cons.tile([128, HID], F32R); nc.sync.dma_start(W1b[:], d_W1[128:256, :])
        W1s = cons.tile([LS, HID], F32R); nc.sync.dma_start(W1s[:], d_W1[256:262, :])
        W2a = cons.tile([128, HID], F32R); nc.sync.dma_start(W2a[:], d_W2[0:128, :])
        W2b = cons.tile([128, HID], F32R); nc.sync.dma_start(W2b[:], d_W2[128:256, :])
        W3a = cons.tile([128, HID], F32R); nc.sync.dma_start(W3a[:], d_W3[0:128, :])
        W3b = cons.tile([128, HID], F32R); nc.sync.dma_start(W3b[:], d_W3[128:256, :])
        Woa = cons.tile([128, ODIM], F32R); nc.sync.dma_start(Woa[:], d_Wo[0:128, :])
        Wob = cons.tile([128, ODIM], F32R); nc.sync.dma_start(Wob[:], d_Wo[128:256, :])
        b1a = cons.tile([128, 1], F32); nc.sync.dma_start(b1a[:], d_b1[0:128, :])
        b1b = cons.tile([128, 1], F32); nc.sync.dma_start(b1b[:], d_b1[128:256, :])
        b2a = cons.tile([128, 1], F32); nc.sync.dma_start(b2a[:], d_b2[0:128, :])
        b2b = cons.tile([128, 1], F32); nc.sync.dma_start(b2b[:], d_b2[128:256, :])
        b3a = cons.tile([128, 1], F32); nc.sync.dma_start(b3a[:], d_b3[0:128, :])
        b3b = cons.tile([128, 1], F32); nc.sync.dma_start(b3b[:], d_b3[128:256, :])

        out_all = [outp.tile([ODIM * NO2, NB], F32, tag=f"oall{h}", name=f"oall{h}")
                   for h in range(2)]

        eng_alt = [nc.scalar, nc.vector]

        def copy_out(eng, dst, src):
            if eng is nc.scalar:
                eng.activation(dst, src, AF.Copy)
            else:
                eng.tensor_copy(dst, src)

        # =================== preprocessing ===================
        with tc.tile_pool(name="zs", bufs=1) as zsp, \
             tc.tile_pool(name="bp", bufs=1) as bpp, \
             tc.tile_pool(name="segsb", bufs=1) as segsb_p, \
             tc.tile_pool(name="znat", bufs=4) as znat_p, \
             tc.tile_pool(name="prod", bufs=7) as prod_p, \
             tc.tile_pool(name="tp_ps", bufs=4, space="PSUM") as tp_ps, \
             tc.tile_pool(name="bp_ps", bufs=2, space="PSUM") as bp_ps, \
             tc.tile_pool(name="seg_ps", bufs=2, space="PSUM") as seg_ps:

            zs = [[zsp.tile([128, NB], F32R, tag=f"zs{c}_{j}", name=f"zs{c}_{j}")
                   for j in range(NT)]
                  for c in range(D)]
            bp = [[bpp.tile([128, NB], F32R, tag=f"bp{c}_{j}", name=f"bp{c}_{j}")
                   for j in range(NT)]
                  for c in range(D)]


            # ---- load z natural + transpose to [L, batch] per channel ----
            for bt in range(NBT):
                znat = znat_p.tile([128, LEN_NOISE * D], F32)
                nc.sync.dma_start(znat[:], d_z[128 * bt:128 * (bt + 1), :])
                for c in range(D):
                    for j in range(NT):
                        ps = tp_ps.tile([128, 128], F32, tag="tp")
                        off = D * (1 + 128 * j) + c
                        nc.tensor.transpose(
                            ps[0:cnt[j], :],
                            znat[:, off:off + D * (cnt[j] - 1) + 1:D],
                            ident[:])
                        copy_out(eng_alt[(c + j) % 2],
                                 zs[c][j][0:cnt[j], 128 * bt:128 * (bt + 1)],
                                 ps[0:cnt[j], :])

            # ---- bp[j] = colsum(zs[<j]) + strict_cumsum(zs[j]) ----
            for c in range(D):
                for j in range(NT):
                    ps = bp_ps.tile([128, NB], F32, tag="bpps")
                    for jp in range(j):
                        nc.tensor.matmul(ps[:], sONES[0:cnt[jp], :],
                                         zs[c][jp][0:cnt[jp], :],
                                         start=(jp == 0), stop=False)
                    nc.tensor.matmul(ps[:], sTS[0:cnt[j], :], zs[c][j][0:cnt[j], :],
                                     start=(j == 0), stop=True)
                    copy_out(eng_alt[j % 2], bp[c][j][:], ps[:])

            # ---- level-1 + level-2 segment sums -> sig scratch in DRAM ----
            zf32 = cons.tile([128, NB], F32)
            nc.vector.memset(zf32[:], 0.0)
            zero6 = cons.tile([LS, NB], F32R)
            nc.scalar.activation(zero6[:], zf32[0:LS, :], AF.Copy)
            nc.sync.dma_start(d_sig[:, 0:NB], zero6[:])

            for c in range(D):
                ps = seg_ps.tile([K, NB], F32, tag="seg")
                for j in range(NT):
                    nc.tensor.matmul(ps[:], sPSEL[0:cnt[j], K * j:K * (j + 1)],
                                     zs[c][j][0:cnt[j], :],
                                     start=(j == 0), stop=(j == NT - 1))
                sb = segsb_p.tile([K, NB], F32R, tag=f"seg{c}")
                nc.scalar.activation(sb[:], ps[:], AF.Copy, scale=sqrt_dt)
                nc.sync.dma_start(
                    d_sig[c:c + 1, NB:].rearrange("s (k n) -> (s k) n", k=K), sb[:])

            for ip, (a, b) in enumerate(PAIRS):
                ps = seg_ps.tile([K, NB], F32, tag="seg")
                for j in range(NT):
                    cj = cnt[j]
                    q1 = prod_p.tile([128, NB], F32R, tag="q")
                    nc.vector.tensor_mul(q1[0:cj, :], bp[a][j][0:cj, :],
                                         zs[b][j][0:cj, :])
                    q2 = prod_p.tile([128, NB], F32R, tag="q")
                    nc.vector.tensor_mul(q2[0:cj, :], bp[b][j][0:cj, :],
                                         zs[a][j][0:cj, :])
                    anti = prod_p.tile([128, NB], F32R, tag="anti")
                    nc.vector.tensor_sub(anti[0:cj, :], q1[0:cj, :], q2[0:cj, :])
                    nc.tensor.matmul(ps[:], sSSEL[0:cnt[j], K * j:K * (j + 1)],
                                     anti[0:cnt[j], :],
                                     start=(j == 0), stop=(j == NT - 1))
                sb = segsb_p.tile([K, NB], F32R, tag=f"seg{3 + ip}")
                nc.scalar.activation(sb[:], ps[:], AF.Copy,
                                     scale=0.5 * sqrt_dt * sqrt_dt)
                nc.sync.dma_start(
                    d_sig[3 + ip:4 + ip, NB:].rearrange("s (k n) -> (s k) n", k=K),
                    sb[:])

        # =================== RNN ===================
        with tc.tile_pool(name="hst", bufs=1) as hst_p, \
             tc.tile_pool(name="sigt", bufs=6) as sigt_p, \
             tc.tile_pool(name="u", bufs=4) as u_p, \
             tc.tile_pool(name="h3", bufs=6) as h3_p, \
             tc.tile_pool(name="dvet", bufs=6) as dvet_p, \
             tc.tile_pool(name="osb", bufs=4) as osb_p, \
             tc.tile_pool(name="p1", bufs=3, space="PSUM") as p1_p, \
             tc.tile_pool(name="p2", bufs=2, space="PSUM") as p2_p, \
             tc.tile_pool(name="p3", bufs=2, space="PSUM") as p3_p, \
             tc.tile_pool(name="po", bufs=1, space="PSUM") as po_p:

            zf32b = hst_p.tile([128, NB], F32, tag="zf32b")
            nc.vector.memset(zf32b[:], 0.0)
            h0a = hst_p.tile([128, NB], F32R, tag="h0a")
            h0b = hst_p.tile([128, NB], F32R, tag="h0b")
            nc.scalar.activation(h0a[:], zf32b[:], AF.Copy)
            nc.scalar.activation(h0b[:], zf32b[:], AF.Copy)
            state = (h0a, h0b)
            latch = C["latch"]

            def prelu_act(dst, psrc, bias_t, alpha):
                nc.scalar.activation(dst, psrc, AF.Prelu, bias=bias_t, alpha=alpha)

            def prelu_dve(dst, psrc, bias_t, alpha):
                t1 = dvet_p.tile([128, NB], F32, tag="t1")
                nc.vector.tensor_scalar_add(t1[:], psrc, bias_t)
                nc.vector.scalar_tensor_tensor(dst, t1[:], alpha, t1[:],
                                               op0=ALU.mult, op1=ALU.max)

            # Emission order: within each latch group, emit the latch step
            # (the group's last step) FIRST so its tanh->L1 critical chain
            # starts immediately; the group's leaf steps fill PE bubbles.
            groups, cur = [], []
            for t in range(n_lags):
                cur.append(t)
                if latch[t]:
                    groups.append(cur)
                    cur = []
            if cur:
                groups.append(cur)
            ordered = []
            for grp in groups:
                if latch[grp[-1]]:
                    ordered += [grp[-1]] + grp[:-1]
                else:
                    ordered += grp

            h3_latch = {}
            for t in ordered:
                ha, hb = state
                sigt = sigt_p.tile([LS, NB], F32R, tag="sigt")
                nc.sync.dma_start(sigt[:], d_sig[:, t * NB:(t + 1) * NB])

                u1 = []
                for i in range(2):
                    ps = p1_p.tile([128, NB], F32, tag="p1")
                    cs = slice(128 * i, 128 * (i + 1))
                    nc.tensor.matmul(ps[:], W1a[:, cs], ha[:], start=True, stop=False)
                    nc.tensor.matmul(ps[:], W1b[:, cs], hb[:], start=False, stop=False)
                    nc.tensor.matmul(ps[:], W1s[:, cs], sigt[:], start=False, stop=True)
                    u = u_p.tile([128, NB], F32R, tag="u1")
                    if i == 0:
                        prelu_act(u[:], ps[:], b1a[:, 0:1], alpha1)
                    else:
                        prelu_dve(u[:], ps[:], b1b[:, 0:1], alpha1)
                    u1.append(u)

                u2 = []
                for i in range(2):
                    ps = p2_p.tile([128, NB], F32, tag="p2")
                    cs = slice(128 * i, 128 * (i + 1))
                    nc.tensor.matmul(ps[:], W2a[:, cs], u1[0][:], start=True, stop=False)
                    nc.tensor.matmul(ps[:], W2b[:, cs], u1[1][:], start=False, stop=True)
                    u = u_p.tile([128, NB], F32R, tag="u2")
                    if i == 0:
                        prelu_act(u[:], ps[:], b2a[:, 0:1], alpha2)
                    else:
                        prelu_dve(u[:], ps[:], b2b[:, 0:1], alpha2)
                    u2.append(u)

                h3 = []
                for i in range(2):
                    ps = p3_p.tile([128, NB], F32, tag="p3")
                    cs = slice(128 * i, 128 * (i + 1))
                    nc.tensor.matmul(ps[:], W3a[:, cs], u2[0][:], start=True, stop=False)
                    nc.tensor.matmul(ps[:], W3b[:, cs], u2[1][:], start=False, stop=True)
                    h = h3_p.tile([128, NB], F32R, tag="h3")
                    nc.scalar.activation(h[:], ps[:], AF.Tanh,
                                         bias=(b3a if i == 0 else b3b)[:, 0:1])
                    h3.append(h)

                ps = po_p.tile([ODIM, NB], F32, tag="po")
                nc.tensor.matmul(ps[:], Woa[:], h3[0][:], start=True, stop=False)
                nc.tensor.matmul(ps[:], Wob[:], h3[1][:], start=False, stop=True)
                osb = osb_p.tile([ODIM, NB], F32, tag="osb")
                nc.scalar.activation(osb[:], ps[:], AF.Copy)
                half, tm = divmod(t, NO2)
                nc.sync.dma_start(out_all[half][ODIM * tm:ODIM * (tm + 1), :],
                                  osb[:])

                if latch[t]:
                    h3_latch[t] = (h3[0], h3[1])
                gi = next(i for i, g in enumerate(groups) if t in g)
                if t == (groups[gi][-2] if len(groups[gi]) > 1 and
                         latch[groups[gi][-1]] else groups[gi][-1]):
                    lt = groups[gi][-1]
                    if latch[lt]:
                        state = h3_latch[lt]

        # ---- transpose outputs to batch-major and store ----
        with tc.tile_pool(name="ot_ps", bufs=2, space="PSUM") as ot_ps, \
             tc.tile_pool(name="otp", bufs=2) as ot_p:
            nrow = ODIM * NO2
            for bt in range(NBT):
                oT = ot_p.tile([128, ODIM * n_lags], F32, tag="oT")
                for half in range(2):
                    ps = ot_ps.tile([128, nrow], F32, tag="ot")
                    nc.tensor.transpose(ps[:, 0:nrow],
                                        out_all[half][0:nrow,
                                                      128 * bt:128 * (bt + 1)],
                                        ident[0:nrow, 0:nrow])
                    nc.scalar.activation(
                        oT[:, half * nrow:(half + 1) * nrow], ps[:, 0:nrow], AF.Copy)
                nc.sync.dma_start(d_out[128 * bt:128 * (bt + 1), :], oT[:])

    nc.finalize()
    return nc, C


def _get(NB, n_lags, alpha1, alpha2):
    key = (NB, n_lags, alpha1, alpha2)
    if key not in _CACHE:
        _CACHE[key] = _build(NB, n_lags, alpha1, alpha2)
    return _CACHE[key]


def run(trace=False, **inputs):
    z = np.ascontiguousarray(np.asarray(inputs["z"], dtype=np.float32))
    B, L, d = z.shape
    n_lags = int(np.asarray(inputs["n_lags"]))
    if not (2 <= n_lags <= 512):
        n_lags = 64
    alpha1 = float(np.asarray(inputs["a1"]).reshape(-1)[0])
    alpha2 = float(np.asarray(inputs["a2"]).reshape(-1)[0])
    NB = B // NCORES
    nc, C = _get(NB, n_lags, alpha1, alpha2)

    f32 = np.float32
    common = {
        "W1": np.ascontiguousarray(np.asarray(inputs["W1"], f32)),
        "W2": np.ascontiguousarray(np.asarray(inputs["W2"], f32)),
        "W3": np.ascontiguousarray(np.asarray(inputs["W3"], f32)),
        "Wo": np.ascontiguousarray(np.asarray(inputs["Wout"], f32)),
        "b1": np.asarray(inputs["b1"], f32).reshape(HID, 1),
        "b2": np.asarray(inputs["b2"], f32).reshape(HID, 1),
        "b3": np.asarray(inputs["b3"], f32).reshape(HID, 1),
        "TS": C["TS"], "ONES": C["ONES"],
        "PSEL": C["PSEL"], "SSEL": C["SSEL"],
    }
    in_maps = []
    for i in range(NCORES):
        m = dict(common)
        m["z"] = z[NB * i:NB * (i + 1)].reshape(NB, L * d)
        in_maps.append(m)

    res = run_bass_kernel_spmd(nc, in_maps, core_ids=list(range(NCORES)),
                               trace=trace)
    out = np.concatenate([r["out"].reshape(NB, n_lags, ODIM)
                          for r in res.results], axis=0)
    return out.astype(np.float32), res


def kernel(**inputs):
    out, _ = run(trace=False, **inputs)
    return out
